# revision 43
# baseline (speedup 1.0000x reference)
"""MelSpectrogram Trainium2 kernel (8-core data-parallel over batch).

Full inputs in, full output out. Host does reflect-pad + layout blocking +
fp32->fp32r rounding; the device does the windowed-DFT matmul (fp32r at full
PE rate), magnitude, mel projection, and log-clip.

Math notes:
  - padded length T+2*PAD = 262912 = 128*2054 exactly; with the SBUF layout
    A[p, j] = padded[128*j + p], the DFT matmul rhs for contraction chunk k
    and frame chunk t0 is the stride-2 free-dim slice A[:, 1024*t0+k :: 2].
  - forward_basis rows [0:513]=real, [513:1026]=imag of bins 0..512. The mel
    matrix has exactly-zero columns at bins 0 and 512 (DC/Nyquist), so the
    kernel computes only bins 0..511 (real) x 0..511 (imag; bin-0 imag is a
    zero row) packed into 1024 = 8 chunks of 128 rows, and contracts mel over
    bins 0..511.
"""

import os

import numpy as np

B, T = 32, 262144
N_FFT = 1024
HOP = 256
N_MELS = 80
CUTOFF = 513
PAD = (N_FFT - HOP) // 2  # 384
TPAD = T + 2 * PAD  # 262912
NCOL = TPAD // 128  # 2054
N_FRAMES = (TPAD - N_FFT) // HOP + 1  # 1024
N_CORES = 8
B_PER_CORE = B // N_CORES  # 4
LOG_CLIP = 1e-5
JUNK1 = int(os.environ.get("K_JUNK1", "11"))
JUNK2 = int(os.environ.get("K_JUNK2", "20"))
TCH = 512  # frames per t-chunk
N_TCH = N_FRAMES // TCH  # 2


def _round_f32r(a):
    """Round fp32 array to fp32r (e8m13) bit pattern, round-to-nearest."""
    b = np.ascontiguousarray(a, dtype=np.float32).view(np.uint32)
    b = (b + np.uint32(0x200)) & np.uint32(0xFFFFFC00)
    return b.view(np.float32)


_PROG = None  # cached (nc, names) across calls


def _build_program():
    import concourse.mybir as mybir
    import concourse.tile as tile
    from concourse import bacc

    F32 = mybir.dt.float32
    F32R = mybir.dt.float32r
    BF16 = mybir.dt.bfloat16
    AF = mybir.ActivationFunctionType

    nc = bacc.Bacc("TRN2", target_bir_lowering=False, debug=False,
                   num_devices=N_CORES)
    audio_d = nc.dram_tensor("audio_blk", [B_PER_CORE, 128, NCOL], F32R,
                             kind="ExternalInput")
    basis_d = nc.dram_tensor("basis_t", [8, 128, 1024], F32R,
                             kind="ExternalInput")
    mel_d = nc.dram_tensor("mel_t", [4, 128, N_MELS], F32R,
                           kind="ExternalInput")
    out_d = nc.dram_tensor("out", [B_PER_CORE, N_MELS, N_FRAMES], F32,
                           kind="ExternalOutput")

    with tile.TileContext(nc) as tc:
        with (
            tc.tile_pool(name="const", bufs=1) as const_pool,
            tc.tile_pool(name="audio", bufs=2) as audio_pool,
            tc.tile_pool(name="sq", bufs=6) as sq_pool,
            tc.tile_pool(name="mag", bufs=B_PER_CORE * N_TCH * 4) as mag_pool,
            tc.tile_pool(name="clip", bufs=B_PER_CORE * N_TCH) as clip_pool,
            tc.tile_pool(name="lnout", bufs=2) as ln_pool,
            tc.tile_pool(name="ftps", bufs=4, space="PSUM") as ft_psum,
        ):
            # Junk-warmup operands; DVE memset (no gpsimd Q7 launch delay).
            junk_w = const_pool.tile([128, 128], F32, tag="junkw")
            nc.vector.memset(junk_w[:], 0.0)
            junk_wb = junk_w.bitcast(BF16)  # [128, 256] bf16 view

            # DMA order matters (the DMA stream serializes): first-needed
            # data first. Audio half 0 of batch 0, then basis chunks in k
            # order, then the rest.
            AH = 1030  # audio halves split; t0=0 needs cols <= 1029
            a_sb0 = audio_pool.tile([128, NCOL], F32R, tag="audio")
            nc.sync.dma_start(a_sb0[:, 0:AH], audio_d[0][:, 0:AH])
            basis_tiles = []
            for k in range(8):
                bt = const_pool.tile([128, 1024], F32R, tag=f"basis{k}")
                if k == 0:
                    nc.sync.dma_start(bt[:, 0:512], basis_d[k][:, 0:512])
                    nc.sync.dma_start(bt[:, 512:1024], basis_d[k][:, 512:1024])
                else:
                    nc.sync.dma_start(bt[:], basis_d[k])
                basis_tiles.append(bt)
            nc.sync.dma_start(a_sb0[:, AH:NCOL], audio_d[0][:, AH:NCOL])
            mel_sb = const_pool.tile([128, 4 * N_MELS], F32R)
            for a in range(4):
                nc.sync.dma_start(mel_sb[:, a * N_MELS:(a + 1) * N_MELS],
                                  mel_d[a])

            # Warmup: ~160 tiny matmuls keep the PE sequencer's run-ahead
            # window occupied while the first DMAs land, so every real
            # matmul issues against a warmed-up tensor engine.
            junk_ps = ft_psum.tile([128, 64], F32, tag="ftr")
            for _ in range(JUNK1):
                # fp32 junk: 4 cycles/row, duration independent of the
                # p-state guess, so the warmup span is guaranteed.
                nc.tensor.matmul(junk_ps[:], junk_w[:],
                                 junk_w[:, 0:64], start=True, stop=True)
            for _ in range(JUNK2):
                # bf16 tail: tiny visit-lag ahead of the first real matmul
                nc.tensor.matmul(junk_ps[:, 0:16], junk_wb[:, 0:128],
                                 junk_wb[:, 0:16], start=True, stop=True)

            sqrt_insts = []

            def mag_chain(a, ftr, fti, mags, splits=1):
                # splits>1 narrows each elementwise op (shorter exposed
                # latency chain; used for the final unit's tail).
                w = TCH // splits
                mag = mag_pool.tile([128, TCH], F32R)
                for h in range(splits):
                    sl = slice(w * h, w * (h + 1))
                    sq_r = sq_pool.tile([128, w], F32, tag="sq")
                    nc.scalar.square(sq_r[:], ftr[:, sl])
                    sq_i = sq_pool.tile([128, w], F32, tag="sq")
                    nc.scalar.square(sq_i[:], fti[:, sl])
                    ssum = sq_pool.tile([128, w], F32, tag="sq")
                    nc.vector.tensor_add(ssum[:], sq_r[:], sq_i[:])
                    sqrt_insts.append(nc.scalar.sqrt(mag[:, sl], ssum[:]))
                mags[a] = mag

            def rhs_ap(a_sb, t0, k):
                st = 2 * TCH * t0 + k
                return a_sb[:, st:st + 2 * TCH - 1:2]

            def emit_unit0(a_sb):
                # k-outer over all 8 (pair, half) groups: consumes basis
                # chunk k over ~1.7us, pacing the PE behind the basis DMA
                # stream with no engine gap. Uses all 8 psum banks.
                mags = [None] * 4
                groups = []
                for a in range(4):
                    ftr = ft_psum.tile([128, TCH], F32, tag="ftr")
                    fti = ft_psum.tile([128, TCH], F32, tag="fti")
                    groups.append((a, ftr, fti))
                for k in range(8):
                    rhs = rhs_ap(a_sb, 0, k)
                    for half in (0, 1):
                        for a, ftr, fti in groups:
                            ft = ftr if half == 0 else fti
                            c0 = 128 * (a + 4 * half)
                            nc.tensor.matmul(
                                ft[:], basis_tiles[k][:, c0:c0 + 128], rhs,
                                start=(k == 0), stop=(k == 7))
                for a, ftr, fti in groups:
                    mag_chain(a, ftr, fti, mags)
                return mags

            def emit_unit(a_sb, t0, splits=1, mid_cb=None):
                # Steady state: k-inner per (real,imag) pair; psum release
                # staggers so the PE never waits.
                mags = [None] * 4
                for a in range(4):
                    if a == 1 and mid_cb is not None:
                        mid_cb()
                    ftr = ft_psum.tile([128, TCH], F32, tag="ftr")
                    fti = ft_psum.tile([128, TCH], F32, tag="fti")
                    for half, ft in ((0, ftr), (1, fti)):
                        for k in range(8):
                            c0 = 128 * (a + 4 * half)
                            nc.tensor.matmul(
                                ft[:], basis_tiles[k][:, c0:c0 + 128],
                                rhs_ap(a_sb, t0, k),
                                start=(k == 0), stop=(k == 7))
                    mag_chain(a, ftr, fti, mags, splits=splits)
                return mags

            clips = []

            def emit_mel(unit, splits=1):
                b, t0, mags = unit
                w = TCH // splits
                for h in range(splits):
                    sl = slice(w * h, w * (h + 1))
                    mel_ps = ft_psum.tile([N_MELS, w], F32, tag="ftr")
                    for a in range(4):
                        nc.tensor.matmul(
                            mel_ps[:], mel_sb[:, a * N_MELS:(a + 1) * N_MELS],
                            mags[a][:, sl], start=(a == 0), stop=(a == 3))
                    ct = clip_pool.tile([N_MELS, w], F32, tag="ct")
                    nc.vector.tensor_scalar_max(ct[:], mel_ps[:], LOG_CLIP)
                    clips.append((b, t0, ct, slice(TCH * t0 + w * h,
                                                   TCH * t0 + w * (h + 1))))

            # ft units with mel pipelined one unit behind (mel matmuls slot
            # into the PE stream only when their mags are long ready).
            unit_sqrt_end = []
            pending = None
            a_sb = a_sb0
            for b in range(B_PER_CORE):
                if b > 0:
                    a_sb = audio_pool.tile([128, NCOL], F32R, tag="audio")
                    nc.sync.dma_start(a_sb[:], audio_d[b])
                for t0 in range(N_TCH):
                    if b == 0 and t0 == 0:
                        mags = emit_unit0(a_sb)
                    else:
                        mags = emit_unit(a_sb, t0)
                    unit_sqrt_end.append(len(sqrt_insts))
                    if pending is not None:
                        emit_mel(pending)
                    pending = (b, t0, mags)
            emit_mel(pending)

            # Ln calls batched in two groups, ordered (same-engine dep) after
            # the sqrt stream so the ACT table set switches at most 4 times,
            # always off the kernel tail's PE-critical path.
            from concourse.tile_rust import add_dep_helper
            gate1 = sqrt_insts[unit_sqrt_end[5] - 1]
            gate2 = sqrt_insts[-1]
            ln_insts = []
            for i, (b, t0, ct, osl) in enumerate(clips):
                lt = ln_pool.tile([N_MELS, osl.stop - osl.start], F32,
                                  tag="lt")
                ln_inst = nc.scalar.activation(lt[:], ct[:], AF.Ln)
                ln_insts.append(ln_inst)
                gate = gate1 if i <= 5 else gate2
                add_dep_helper(ln_inst.ins, gate.ins, sync=False,
                               reason="ln ordered after sqrt batch")
                nc.sync.dma_start(out_d[b][:, osl], lt[:])
            for si in range(unit_sqrt_end[5], len(sqrt_insts)):
                add_dep_helper(sqrt_insts[si].ins, ln_insts[5].ins, sync=False,
                               reason="sqrt batch2 after ln batch1")

    nc.compile()
    return nc


def _get_program():
    global _PROG
    if _PROG is None:
        _PROG = _build_program()
    return _PROG


def _prep_inputs(audio, forward_basis, mel_basis):
    audio = np.asarray(audio, dtype=np.float32)
    fb = np.asarray(forward_basis, dtype=np.float32)
    mel = np.asarray(mel_basis, dtype=np.float32)
    assert audio.shape == (B, T)
    assert fb.shape == (2 * CUTOFF, N_FFT)
    assert mel.shape == (N_MELS, CUTOFF)

    padded = np.pad(audio, ((0, 0), (PAD, PAD)), mode="reflect")
    # A[b, p, j] = padded[b, 128*j + p]
    blk = _round_f32r(
        np.ascontiguousarray(padded.reshape(B, NCOL, 128).transpose(0, 2, 1)))

    # basisP rows: [real bins 0..511, imag bins 0..511]; basis_t[k,p,c] =
    # basisP[c, 128k+p]
    basisP = np.concatenate([fb[0:512], fb[513:1025]], axis=0)  # [1024, 1024]
    basis_t = _round_f32r(
        np.ascontiguousarray(basisP.T).reshape(8, 128, 1024))

    mel_t = _round_f32r(
        np.ascontiguousarray(mel[:, 0:512].T).reshape(4, 128, N_MELS))
    return blk, basis_t, mel_t


def kernel(audio, forward_basis=None, mel_basis=None, jitter_steps=0,
           **_unused):
    from concourse.bass_utils import run_bass_kernel_spmd

    if forward_basis is None or mel_basis is None:
        fb, mel = _default_bases()
        forward_basis = fb if forward_basis is None else forward_basis
        mel_basis = mel if mel_basis is None else mel_basis

    blk, basis_t, mel_t = _prep_inputs(audio, forward_basis, mel_basis)
    nc = _get_program()
    in_maps = [
        {"audio_blk": blk[i * B_PER_CORE:(i + 1) * B_PER_CORE],
         "basis_t": basis_t, "mel_t": mel_t}
        for i in range(N_CORES)
    ]
    res = run_bass_kernel_spmd(nc, in_maps, core_ids=list(range(N_CORES)),
                               trace=False)
    out = np.concatenate([res.results[i]["out"] for i in range(N_CORES)],
                         axis=0)
    return out.astype(np.float32)


def _default_bases():
    """Recompute forward/mel bases (deterministic) if not passed in."""
    n = N_FFT
    hann = (0.5 - 0.5 * np.cos(2.0 * np.pi * np.arange(n) / n))
    fourier = np.fft.fft(np.eye(n))
    fb = np.vstack([np.real(fourier[:CUTOFF, :]), np.imag(fourier[:CUTOFF, :])])
    fb = (fb * hann[None, :]).astype(np.float32)

    def hz_to_mel(f):
        f = np.atleast_1d(np.asarray(f, dtype=np.float64))
        f_sp = 200.0 / 3
        mels = f / f_sp
        min_log_hz = 1000.0
        min_log_mel = min_log_hz / f_sp
        logstep = np.log(6.4) / 27.0
        log_t = f >= min_log_hz
        mels[log_t] = min_log_mel + np.log(f[log_t] / min_log_hz) / logstep
        return mels

    def mel_to_hz(m):
        m = np.asarray(m, dtype=np.float64)
        f_sp = 200.0 / 3
        freqs = m * f_sp
        min_log_hz = 1000.0
        min_log_mel = min_log_hz / f_sp
        logstep = np.log(6.4) / 27.0
        log_t = m >= min_log_mel
        return np.where(log_t, min_log_hz * np.exp(logstep * (m - min_log_mel)),
                        freqs)

    sr, n_mels, fmin = 22050, N_MELS, 0.0
    fmax = sr / 2.0
    mel_f = mel_to_hz(np.linspace(hz_to_mel(fmin)[0], hz_to_mel(fmax)[0],
                                  n_mels + 2))
    fftfreqs = np.linspace(0.0, sr / 2.0, CUTOFF)
    fdiff = np.diff(mel_f)
    ramps = mel_f[:, None] - fftfreqs[None, :]
    lower = -ramps[:-2] / fdiff[:-1, None]
    upper = ramps[2:] / fdiff[1:, None]
    weights = np.maximum(0.0, np.minimum(lower, upper))
    enorm = 2.0 / (mel_f[2:n_mels + 2] - mel_f[:n_mels])
    weights *= enorm[:, None]
    return fb, weights.astype(np.float32)
